# revision 1
# baseline (speedup 1.0000x reference)
"""Self-contained Trainium2 Bass kernel for the ragged centroid L1 loss.

Math per sample b (L = unit_lengths[b], D = 1024):
    G    = C[units[b, :L]]                    # (L, D) codebook row gather
    CT   = centroids[b, :L, :].T              # (D, L)
    true = G.reshape(D, L)                    # row-major reshape (flat pairing)
    loss_b = np.abs(CT - true).sum() / L
    out = mean_b(loss_b)

Key identity: CT.flat[m] pairs with G.flat[m] for m < D*L, so after
materializing G contiguously in DRAM scratch, the G side of any CT tile
(128 CT rows x t-block) is a clean 2D strided DMA ([stride L, 128] x [1, tw]),
and the CT side is built on-chip by PE-transposing centroid tiles.

Distribution: pure data parallel over B=16 samples, 2 per core on 8 cores
(paired large+small L for balance). Per-sample lengths are compile-time
constants (the program is rebuilt per distinct length multiset; gather
indices stay runtime data via dma_gather). Final mean is reduced on host
from per-partition partial sums.
"""
import sys

sys.path.insert(0, "/opt/trn_rl_repo")

from contextlib import ExitStack

import numpy as np

import concourse.bass as bass
import concourse.tile as tile
from concourse import bacc, masks, mybir
from concourse.bass_utils import run_bass_kernel_spmd

F32 = mybir.dt.float32
I16 = mybir.dt.int16

D = 1024          # feature dim == codebook row length
K = 1024          # codebook rows
T = 4096          # max sequence length
B = 16            # batch
NCORES = 8
SPC = 2           # samples per core
GCH = 512         # gather chunk (rows per dma_gather)
TBLK = 512        # compare tile width along t
NGRP = D // 128   # CT row groups of 128
MAXCH = T // GCH  # 8 chunks max
IDXW = T // 16    # wrapped idx columns (256)


def _emit_sample(tc, nc, pools, aps, slot, L):
    idxp, gatp, centp, gp, psp, dfp, accp, identity, outacc = pools
    cent_in, cmat_in, idx_in, gscr = aps
    gscr_ap = gscr[slot]

    idx_sb = idxp.tile([128, IDXW], I16, tag="idx")
    nc.gpsimd.dma_start(idx_sb[:], idx_in[slot])

    acc = accp.tile([128, 64], F32, tag="acc")
    nc.vector.memset(acc[:], 0.0)

    # phase 1: gather codebook rows -> contiguous G in DRAM scratch.
    # Chunks are padded with index 0 so the full tile is always written.
    nch = -(-L // GCH)
    for c in range(nch):
        gout = gatp.tile([128, GCH // 128, D], F32, tag="gout")
        nc.gpsimd.dma_gather(
            gout[:], cmat_in, idx_sb[:, c * (GCH // 16):(c + 1) * (GCH // 16)],
            GCH, GCH, D,
        )
        dst = gscr_ap[bass.ds(c * GCH * D, GCH * D)].rearrange(
            "(g p c2) -> p g c2", p=128, g=GCH // 128
        )
        nc.gpsimd.dma_start(dst, gout[:])

    # phase 2: per t-block, transpose centroid tiles once and compare
    # against all 8 CT row groups.
    nt = -(-L // TBLK)
    col = 0
    for tb in range(nt):
        t0 = tb * TBLK
        tw = min(TBLK, L - t0)
        nk = -(-tw // 128)
        cts = []
        for k in range(nk):
            wt = min(128, tw - k * 128)
            ct = centp.tile([wt, D], F32, tag="ct")
            nc.sync.dma_start(ct[:], cent_in[slot, t0 + k * 128:t0 + k * 128 + wt, :])
            cts.append((ct, wt))
        for g in range(NGRP):
            gg = gp.tile([128, tw], F32, tag="gg")
            gsl = gscr_ap[bass.ds(g * 128 * L, 128 * L)].rearrange(
                "(p t) -> p t", p=128
            )[:, t0:t0 + tw]
            nc.scalar.dma_start(gg[:], gsl)
            ps = psp.tile([128, tw], F32, tag="ps")
            for k, (ct, wt) in enumerate(cts):
                nc.tensor.transpose(
                    ps[:, k * 128:k * 128 + wt],
                    ct[:, g * 128:(g + 1) * 128],
                    identity[:wt, :wt],
                )
            df = dfp.tile([128, tw], F32, tag="df")
            nc.vector.tensor_sub(df[:], ps[:], gg[:])
            nc.vector.tensor_reduce(
                acc[:, col:col + 1], df[:], mybir.AxisListType.X,
                mybir.AluOpType.add, apply_absolute_value=True,
            )
            col += 1
    nc.vector.tensor_reduce(
        outacc[:, slot:slot + 1], acc[:, 0:col], mybir.AxisListType.X,
        mybir.AluOpType.add,
    )


def _build(core_lengths):
    """core_lengths: list of NCORES tuples (L_slot0, L_slot1)."""
    nc = bacc.Bacc("TRN2", target_bir_lowering=False, debug=False,
                   num_devices=NCORES)
    cent_in = nc.dram_tensor("cent", [SPC, T, D], F32, kind="ExternalInput").ap()
    cmat_in = nc.dram_tensor("cmat", [K, D], F32, kind="ExternalInput").ap()
    idx_in = nc.dram_tensor("idx", [SPC, 128, IDXW], I16, kind="ExternalInput").ap()
    out_d = nc.dram_tensor("out", [128, SPC], F32, kind="ExternalOutput").ap()
    gscr = [nc.dram_tensor(f"gscr{s}", [T * D], F32).ap() for s in range(SPC)]

    with tile.TileContext(nc) as tc, ExitStack() as ctx:
        idxp = ctx.enter_context(tc.tile_pool(name="idx", bufs=2))
        gatp = ctx.enter_context(tc.tile_pool(name="gat", bufs=3))
        centp = ctx.enter_context(tc.tile_pool(name="cent", bufs=8))
        gp = ctx.enter_context(tc.tile_pool(name="g", bufs=4))
        psp = ctx.enter_context(tc.tile_pool(name="ps", bufs=4, space="PSUM"))
        dfp = ctx.enter_context(tc.tile_pool(name="df", bufs=4))
        accp = ctx.enter_context(tc.tile_pool(name="acc", bufs=2))
        outp = ctx.enter_context(tc.tile_pool(name="outacc", bufs=1))
        identp = ctx.enter_context(tc.tile_pool(name="ident", bufs=1))

        identity = identp.tile([128, 128], F32)
        masks.make_identity(nc, identity[:])
        pid = nc.partition_id()

        for core in range(NCORES):
            with tc.If(pid == core):
                outacc = outp.tile([128, SPC], F32, tag="oacc")
                pools = (idxp, gatp, centp, gp, psp, dfp, accp, identity, outacc)
                aps = (cent_in, cmat_in, idx_in, gscr)
                for slot in range(SPC):
                    _emit_sample(tc, nc, pools, aps, slot, core_lengths[core][slot])
                nc.sync.dma_start(out_d, outacc[:])
    nc.compile()
    return nc


_CACHE = {}


def _get_program(core_lengths):
    key = tuple(core_lengths)
    if key not in _CACHE:
        _CACHE[key] = _build(core_lengths)
    return _CACHE[key]


def _plan(unit_lengths):
    """Pair samples (largest with smallest) and order pairs heavy-first."""
    order = np.argsort(-unit_lengths, kind="stable")
    pairs = [(int(order[i]), int(order[B - 1 - i])) for i in range(NCORES)]
    pairs.sort(key=lambda p: -(unit_lengths[p[0]] + unit_lengths[p[1]]))
    return pairs


def _wrap_idx(units_row, L):
    """int16 wrapped layout: idx k at [k % 16, k // 16]; pad with 0."""
    arr = np.zeros((128, IDXW), dtype=np.int16)
    v = units_row[:L].astype(np.int16)
    k = np.arange(L)
    arr[k % 16, k // 16] = v
    return arr


def _run(inputs, trace=False, tmpdir=None):
    centroids = np.ascontiguousarray(np.asarray(inputs["centroids"]), dtype=np.float32)
    units = np.asarray(inputs["units"])
    unit_lengths = np.asarray(inputs["unit_lengths"]).astype(np.int64)
    C = np.ascontiguousarray(np.asarray(inputs["C"]), dtype=np.float32)
    assert centroids.shape == (B, T, D) and C.shape == (K, D)

    pairs = _plan(unit_lengths)
    core_lengths = tuple(
        (int(unit_lengths[a]), int(unit_lengths[b])) for a, b in pairs
    )
    nc = _get_program(core_lengths)

    in_maps = []
    for a, b in pairs:
        in_maps.append({
            "cent": np.stack([centroids[a], centroids[b]]),
            "cmat": C,
            "idx": np.stack([
                _wrap_idx(units[a], int(unit_lengths[a])),
                _wrap_idx(units[b], int(unit_lengths[b])),
            ]),
        })

    res = run_bass_kernel_spmd(nc, in_maps, list(range(NCORES)),
                               trace=trace, tmpdir=tmpdir)

    total = 0.0
    for core, (a, b) in enumerate(pairs):
        sums = res.results[core]["out"].astype(np.float64)
        total += sums[:, 0].sum() / float(unit_lengths[a])
        total += sums[:, 1].sum() / float(unit_lengths[b])
    return np.float32(total / B), res


def kernel(**inputs):
    out, _ = _run(inputs)
    return out


# revision 2
# speedup vs baseline: 1.0349x; 1.0349x over previous
"""Self-contained Trainium2 Bass kernel for the ragged centroid L1 loss.

Math per sample b (L = unit_lengths[b], D = 1024):
    G    = C[units[b, :L]]                    # (L, D) codebook row gather
    CT   = centroids[b, :L, :].T              # (D, L)
    true = G.reshape(D, L)                    # row-major reshape (flat pairing)
    loss_b = np.abs(CT - true).sum() / L
    out = mean_b(loss_b)

Key identity: CT.flat[m] pairs with G.flat[m] for m < D*L, so after
materializing G contiguously in DRAM scratch, the G side of any CT tile
(128 CT rows x t-block) is a clean 2D strided DMA ([stride L, 128] x [1, tw]),
and the CT side is built on-chip by PE-transposing centroid tiles.

Distribution: pure data parallel over B=16 samples, 2 per core on 8 cores
(paired large+small L for balance). Per-sample lengths are compile-time
constants (the program is rebuilt per distinct length multiset; gather
indices stay runtime data via dma_gather). Final mean is reduced on host
from per-partition partial sums.
"""
import sys

sys.path.insert(0, "/opt/trn_rl_repo")

from contextlib import ExitStack

import numpy as np

import concourse.bass as bass
import concourse.tile as tile
from concourse import bacc, masks, mybir
from concourse.bass_utils import run_bass_kernel_spmd

F32 = mybir.dt.float32
I16 = mybir.dt.int16

D = 1024          # feature dim == codebook row length
K = 1024          # codebook rows
T = 4096          # max sequence length
B = 16            # batch
NCORES = 8
SPC = 2           # samples per core
GCH = 512         # gather chunk (rows per dma_gather)
TBLK = 512        # compare tile width along t
NGRP = D // 128   # CT row groups of 128
MAXCH = T // GCH  # 8 chunks max
IDXW = T // 16    # wrapped idx columns (256)


def _emit_sample(tc, nc, pools, aps, slot, L):
    idxp, gatp, centp, gp, psp, dfp, accp, identity, outacc = pools
    cent_in, cmat_in, idx_in, gscr = aps
    gscr_ap = gscr[slot]

    idx_sb = idxp.tile([128, IDXW], I16, tag="idx")
    nc.gpsimd.dma_start(idx_sb[:], idx_in[slot])

    acc = accp.tile([128, 64], F32, tag="acc")
    nc.vector.memset(acc[:], 0.0)

    # phase 1: gather codebook rows -> contiguous G in DRAM scratch.
    # Chunks are padded with index 0 so the full tile is always written.
    nch = -(-L // GCH)
    for c in range(nch):
        gout = gatp.tile([128, GCH // 128, D], F32, tag="gout")
        nc.gpsimd.dma_gather(
            gout[:], cmat_in, idx_sb[:, c * (GCH // 16):(c + 1) * (GCH // 16)],
            GCH, GCH, D,
        )
        dst = gscr_ap[bass.ds(c * GCH * D, GCH * D)].rearrange(
            "(g p c2) -> p g c2", p=128, g=GCH // 128
        )
        nc.gpsimd.dma_start(dst, gout[:])

    # phase 2: per t-block, transpose centroid tiles once and compare
    # against all 8 CT row groups.
    nt = -(-L // TBLK)
    col = 0
    for tb in range(nt):
        t0 = tb * TBLK
        tw = min(TBLK, L - t0)
        nk = -(-tw // 128)
        cts = []
        for k in range(nk):
            wt = min(128, tw - k * 128)
            ct = centp.tile([wt, D], F32, tag="ct")
            nc.sync.dma_start(ct[:], cent_in[slot, t0 + k * 128:t0 + k * 128 + wt, :])
            cts.append((ct, wt))
        for g in range(NGRP):
            gg = gp.tile([128, tw], F32, tag="gg")
            gsl = gscr_ap[bass.ds(g * 128 * L, 128 * L)].rearrange(
                "(p t) -> p t", p=128
            )[:, t0:t0 + tw]
            nc.scalar.dma_start(gg[:], gsl)
            ps = psp.tile([128, tw], F32, tag="ps")
            for k, (ct, wt) in enumerate(cts):
                nc.tensor.transpose(
                    ps[:, k * 128:k * 128 + wt],
                    ct[:, g * 128:(g + 1) * 128],
                    identity[:wt, :wt],
                )
            df = dfp.tile([128, tw], F32, tag="df")
            nc.vector.tensor_sub(df[:], ps[:], gg[:])
            nc.vector.tensor_reduce(
                acc[:, col:col + 1], df[:], mybir.AxisListType.X,
                mybir.AluOpType.add, apply_absolute_value=True,
            )
            col += 1
    nc.vector.tensor_reduce(
        outacc[:, slot:slot + 1], acc[:, 0:col], mybir.AxisListType.X,
        mybir.AluOpType.add,
    )


def _build(core_lengths):
    """core_lengths: list of NCORES tuples (L_slot0, L_slot1)."""
    nc = bacc.Bacc("TRN2", target_bir_lowering=False, debug=False,
                   num_devices=NCORES)
    cent_in = nc.dram_tensor("cent", [SPC, T, D], F32, kind="ExternalInput").ap()
    cmat_in = nc.dram_tensor("cmat", [K, D], F32, kind="ExternalInput").ap()
    idx_in = nc.dram_tensor("idx", [SPC, 128, IDXW], I16, kind="ExternalInput").ap()
    out_d = nc.dram_tensor("out", [128, SPC], F32, kind="ExternalOutput").ap()
    gscr = [nc.dram_tensor(f"gscr{s}", [T * D], F32).ap() for s in range(SPC)]

    with tile.TileContext(nc) as tc, ExitStack() as ctx:
        idxp = ctx.enter_context(tc.tile_pool(name="idx", bufs=2))
        gatp = ctx.enter_context(tc.tile_pool(name="gat", bufs=3))
        centp = ctx.enter_context(tc.tile_pool(name="cent", bufs=8))
        gp = ctx.enter_context(tc.tile_pool(name="g", bufs=4))
        psp = ctx.enter_context(tc.tile_pool(name="ps", bufs=4, space="PSUM"))
        dfp = ctx.enter_context(tc.tile_pool(name="df", bufs=4))
        accp = ctx.enter_context(tc.tile_pool(name="acc", bufs=2))
        outp = ctx.enter_context(tc.tile_pool(name="outacc", bufs=1))
        identp = ctx.enter_context(tc.tile_pool(name="ident", bufs=1))

        identity = identp.tile([128, 128], F32)
        masks.make_identity(nc, identity[:])
        pid = nc.partition_id()

        for core in range(NCORES):
            with tc.If(pid == core):
                outacc = outp.tile([128, SPC], F32, tag="oacc")
                pools = (idxp, gatp, centp, gp, psp, dfp, accp, identity, outacc)
                aps = (cent_in, cmat_in, idx_in, gscr)
                for slot in range(SPC):
                    _emit_sample(tc, nc, pools, aps, slot, core_lengths[core][slot])
                nc.sync.dma_start(out_d, outacc[:])
    nc.compile()
    return nc


_CACHE = {}


def _get_program(core_lengths):
    key = tuple(core_lengths)
    if key not in _CACHE:
        _CACHE[key] = _build(core_lengths)
    return _CACHE[key]


def _plan(unit_lengths):
    """Pair samples (largest with smallest) and order pairs heavy-first."""
    order = np.argsort(-unit_lengths, kind="stable")
    pairs = [(int(order[i]), int(order[B - 1 - i])) for i in range(NCORES)]
    pairs.sort(key=lambda p: -(unit_lengths[p[0]] + unit_lengths[p[1]]))
    return pairs


def _wrap_idx(units_row, L):
    """int16 wrapped layout: idx k at [k % 16, k // 16]; pad with 0.

    The 16-partition pattern is replicated to all 8 gpsimd cores
    (partitions 16j..16j+15)."""
    arr = np.zeros((16, IDXW), dtype=np.int16)
    v = units_row[:L].astype(np.int16)
    k = np.arange(L)
    arr[k % 16, k // 16] = v
    return np.tile(arr, (8, 1))


def _run(inputs, trace=False, tmpdir=None):
    centroids = np.ascontiguousarray(np.asarray(inputs["centroids"]), dtype=np.float32)
    units = np.asarray(inputs["units"])
    unit_lengths = np.asarray(inputs["unit_lengths"]).astype(np.int64)
    C = np.ascontiguousarray(np.asarray(inputs["C"]), dtype=np.float32)
    assert centroids.shape == (B, T, D) and C.shape == (K, D)

    pairs = _plan(unit_lengths)
    core_lengths = tuple(
        (int(unit_lengths[a]), int(unit_lengths[b])) for a, b in pairs
    )
    nc = _get_program(core_lengths)

    in_maps = []
    for a, b in pairs:
        in_maps.append({
            "cent": np.stack([centroids[a], centroids[b]]),
            "cmat": C,
            "idx": np.stack([
                _wrap_idx(units[a], int(unit_lengths[a])),
                _wrap_idx(units[b], int(unit_lengths[b])),
            ]),
        })

    res = run_bass_kernel_spmd(nc, in_maps, list(range(NCORES)),
                               trace=trace, tmpdir=tmpdir)

    total = 0.0
    for core, (a, b) in enumerate(pairs):
        sums = res.results[core]["out"].astype(np.float64)
        total += sums[:, 0].sum() / float(unit_lengths[a])
        total += sums[:, 1].sum() / float(unit_lengths[b])
    return np.float32(total / B), res


def kernel(**inputs):
    out, _ = _run(inputs)
    return out


# revision 7
# speedup vs baseline: 1.3082x; 1.2641x over previous
"""Self-contained Trainium2 Bass kernel for the ragged centroid L1 loss.

Math per sample b (L = unit_lengths[b], D = 1024):
    G    = C[units[b, :L]]                    # (L, D) codebook row gather
    CT   = centroids[b, :L, :].T              # (D, L)
    true = G.reshape(D, L)                    # row-major reshape (flat pairing)
    loss_b = np.abs(CT - true).sum() / L
    out = mean_b(loss_b)

Key identity: CT.flat[m] pairs with G.flat[m] for m < D*L. CT row group
g (rows g*128..g*128+127) pairs exactly with the contiguous G.flat range
[g*128*L, (g+1)*128*L), i.e. G rows [g*L/8, (g+1)*L/8] -- so the gather
is split into per-group-range "units" landing in their own DRAM scratch
tensors, making the compare for group g depend only on its own slice of
the gather (pipeline instead of a full-gather bubble). The G side of a
CT tile (128 rows x t-block) is then a clean 2D strided DMA
([stride L, 128] x [1, tw]); the CT side is PE-transposed centroid
tiles consumed directly from PSUM.

Distribution: data parallel over the B=16 samples on 8 cores. Work
splits with zero duplication at CT-row-group granularity, so large
samples are split across cores in units of group-PAIRS (1/4 samples);
a two-phase planner (whole-sample LPT + pair moves off the max core)
balances per-core gathered-row loads. Per-sample lengths are
compile-time constants (program rebuilt per distinct layout; gather
indices stay runtime data via dma_gather). Final mean reduced on host
from per-partition partial sums.
"""
import sys

sys.path.insert(0, "/opt/trn_rl_repo")

from contextlib import ExitStack

import numpy as np

import concourse.bass as bass
import concourse.tile as tile
from concourse import bacc, masks, mybir
from concourse.bass_utils import run_bass_kernel_spmd

F32 = mybir.dt.float32
I16 = mybir.dt.int16

D = 1024          # feature dim == codebook row length
K = 1024          # codebook rows
T = 4096          # max sequence length
B = 16            # batch
NCORES = 8
SLOTS = 4         # max distinct samples (input slots) per core
NGRP = D // 128   # CT row groups of 128
NPAIR = NGRP // 2
GMAX = 640        # max rows per gather unit (>= L/8 + 2 for any L <= 4096)
TBLK = 512        # compare tile width along t
IDXC = GMAX // 16  # wrapped idx columns per unit (40)
# scratch elems per unit: unit rows plus a full 128*T window of slack so
# the strided reload's ds() window never overruns the tensor
GSCR_ELEMS = GMAX * D + 128 * T


def _units_for(L):
    """Partition the 8 CT row groups into gather units.

    Group g needs G rows [floor(g*128*L/1024), ceil((g+1)*128*L/1024)).
    Greedily merge consecutive groups while the union stays <= GMAX rows.
    Returns (units, group_unit): units = list of (row_lo, row_hi),
    group_unit[g] = unit index.
    """
    lo = [(g * 128 * L) // D for g in range(NGRP)]
    hi = [-(-((g + 1) * 128 * L) // D) for g in range(NGRP)]
    units = []
    group_unit = [0] * NGRP
    cur_lo, cur_hi = lo[0], hi[0]
    gs = [0]
    for g in range(1, NGRP):
        if hi[g] - cur_lo <= GMAX:
            cur_hi = hi[g]
            gs.append(g)
        else:
            units.append((cur_lo, cur_hi))
            for gg in gs:
                group_unit[gg] = len(units) - 1
            cur_lo, cur_hi = lo[g], hi[g]
            gs = [g]
    units.append((cur_lo, cur_hi))
    for gg in gs:
        group_unit[gg] = len(units) - 1
    return units, group_unit


def _emit_job(tc, nc, pools, aps, outacc, slot, L, pairs, col0):
    """Emit gather+compare for group pairs `pairs` of one sample.

    Returns the number of acc columns used."""
    idxp, gatp, centp, gp, psp, dfp, accp, identity = pools
    cent_in, cmat_in, idx_in, gscr = aps

    idx_sb = idxp.tile([128, NGRP * IDXC], I16, tag="idx")
    nc.gpsimd.dma_start(idx_sb[:], idx_in[slot])

    acc = accp.tile([128, 64], F32, tag="acc")
    nc.vector.memset(acc[:], 0.0)

    units, group_unit = _units_for(L)
    nt = -(-L // TBLK)
    gathered = set()

    def emit_gather(g):
        u = group_unit[g]
        if u in gathered:
            return
        gathered.add(u)
        rlo, rhi = units[u]
        ng = -(-(rhi - rlo) // 128)
        gout = gatp.tile([128, GMAX // 128, D], F32, tag="gout")
        nc.gpsimd.dma_gather(
            gout[:, 0:ng, :], cmat_in, idx_sb[:, u * IDXC:u * IDXC + ng * 8],
            ng * 128, ng * 128, D,
        )
        dst = gscr[slot][u][bass.ds(0, ng * 128 * D)].rearrange(
            "(g p c) -> p g c", p=128, g=ng
        )
        nc.sync.dma_start(dst, gout[:, 0:ng, :])

    emit_gather(pairs[0] * 2)
    emit_gather(pairs[0] * 2 + 1)

    col = 0
    for pi, pair in enumerate(pairs):
        g0 = pair * 2
        if pi + 1 < len(pairs):
            emit_gather(pairs[pi + 1] * 2)
            emit_gather(pairs[pi + 1] * 2 + 1)
        for tb in range(nt):
            t0 = tb * TBLK
            tw = min(TBLK, L - t0)
            nk = -(-tw // 128)
            wlast = tw - (nk - 1) * 128
            # centroid block: rows [t0, t0+tw), cols [g0*128, (g0+2)*128)
            ct = centp.tile([128, 4, 256], F32, tag="ct")
            if nk > 1:
                nc.sync.dma_start(
                    ct[:, 0:nk - 1, :],
                    cent_in[slot, t0:t0 + (nk - 1) * 128,
                            g0 * 128:(g0 + 2) * 128]
                    .rearrange("(q p) c -> p q c", p=128),
                )
            nc.sync.dma_start(
                ct[0:wlast, nk - 1, :],
                cent_in[slot, t0 + (nk - 1) * 128:t0 + tw,
                        g0 * 128:(g0 + 2) * 128],
            )
            for g in (g0, g0 + 1):
                u = group_unit[g]
                rlo = units[u][0]
                off = g * 128 * L - rlo * D + t0
                gv = gscr[slot][u][bass.ds(off, 128 * L)].rearrange(
                    "(p t) -> p t", p=128
                )[:, 0:tw]
                gg = gp.tile([128, TBLK], F32, tag="gg")
                nc.scalar.dma_start(gg[:, 0:tw], gv)
                ps = psp.tile([128, TBLK], F32, tag="ps")
                for q in range(nk):
                    wt = 128 if q < nk - 1 else wlast
                    src = ct[0:wt, q, (g - g0) * 128:(g - g0 + 1) * 128]
                    nc.tensor.transpose(
                        ps[:, q * 128:q * 128 + wt], src,
                        identity[0:wt, 0:wt],
                    )
                df = dfp.tile([128, TBLK], F32, tag="df")
                nc.vector.tensor_sub(df[:, 0:tw], ps[:, 0:tw], gg[:, 0:tw])
                nc.vector.tensor_reduce(
                    acc[:, col:col + 1], df[:, 0:tw], mybir.AxisListType.X,
                    mybir.AluOpType.add, apply_absolute_value=True,
                )
                col += 1
    nc.vector.tensor_reduce(
        outacc[:, slot:slot + 1], acc[:, 0:col], mybir.AxisListType.X,
        mybir.AluOpType.add,
    )
    return col


def _build(core_jobs):
    """core_jobs: tuple of NCORES tuples of (L, pairs) per slot."""
    nc = bacc.Bacc("TRN2", target_bir_lowering=False, debug=False,
                   num_devices=NCORES)
    cent_in = nc.dram_tensor("cent", [SLOTS, T, D], F32, kind="ExternalInput").ap()
    cmat_in = nc.dram_tensor("cmat", [K, D], F32, kind="ExternalInput").ap()
    idx_in = nc.dram_tensor("idx", [SLOTS, 128, NGRP * IDXC], I16,
                            kind="ExternalInput").ap()
    out_d = nc.dram_tensor("out", [128, SLOTS], F32, kind="ExternalOutput").ap()
    gscr = [[nc.dram_tensor(f"gscr{s}_{u}", [GSCR_ELEMS], F32).ap()
             for u in range(NGRP)] for s in range(SLOTS)]

    with tile.TileContext(nc) as tc, ExitStack() as ctx:
        idxp = ctx.enter_context(tc.tile_pool(name="idx", bufs=2))
        gatp = ctx.enter_context(tc.tile_pool(name="gat", bufs=3))
        centp = ctx.enter_context(tc.tile_pool(name="cent", bufs=4))
        gp = ctx.enter_context(tc.tile_pool(name="g", bufs=4))
        psp = ctx.enter_context(tc.tile_pool(name="ps", bufs=4, space="PSUM"))
        dfp = ctx.enter_context(tc.tile_pool(name="df", bufs=4))
        accp = ctx.enter_context(tc.tile_pool(name="acc", bufs=2))
        outp = ctx.enter_context(tc.tile_pool(name="outacc", bufs=1))
        identp = ctx.enter_context(tc.tile_pool(name="ident", bufs=1))

        identity = identp.tile([128, 128], F32)
        masks.make_identity(nc, identity[:])
        pid = nc.partition_id()

        pools = (idxp, gatp, centp, gp, psp, dfp, accp, identity)
        aps = (cent_in, cmat_in, idx_in, gscr)

        def arm(core):
            outacc = outp.tile([128, SLOTS], F32, tag="oacc")
            nc.vector.memset(outacc[:], 0.0)
            for slot, (L, pairs) in enumerate(core_jobs[core]):
                _emit_job(tc, nc, pools, aps, outacc, slot, L, list(pairs), 0)
            nc.sync.dma_start(out_d, outacc[:])

        # 3-level dispatch tree: each core traverses ~3 branches
        with tc.If(pid < 4) as c0:
            with tc.If(pid < 2) as c1:
                with tc.If(pid < 1) as c2:
                    arm(0)
                with c2.Else():
                    arm(1)
            with c1.Else():
                with tc.If(pid < 3) as c3:
                    arm(2)
                with c3.Else():
                    arm(3)
        with c0.Else():
            with tc.If(pid < 6) as c4:
                with tc.If(pid < 5) as c5:
                    arm(4)
                with c5.Else():
                    arm(5)
            with c4.Else():
                with tc.If(pid < 7) as c6:
                    arm(6)
                with c6.Else():
                    arm(7)
    nc.compile()
    return nc


_CACHE = {}


def _get_program(core_jobs):
    key = tuple(core_jobs)
    if key not in _CACHE:
        _CACHE[key] = _build(core_jobs)
    return _CACHE[key]


def _plan(unit_lengths):
    """Two-phase work assignment.

    Phase 1: whole-sample LPT, 2 samples per core. Phase 2: move single
    group-pairs (1/4 of a sample, zero duplication) from the max-loaded
    core to the least-loaded eligible core. Returns a list of NCORES
    job-lists [(sample, sorted_pairs)], heaviest core first.
    """
    n = len(unit_lengths)
    uls = [int(x) for x in unit_lengths]
    order = sorted(range(n), key=lambda s: -uls[s])
    assign = [dict() for _ in range(NCORES)]  # sample -> set(pairs)
    loads = [0.0] * NCORES
    for s in order:
        c = min((c for c in range(NCORES) if len(assign[c]) < 2),
                key=lambda c: loads[c])
        assign[c][s] = {0, 1, 2, 3}
        loads[c] += uls[s]
    for _ in range(200):
        hi = max(range(NCORES), key=lambda c: loads[c])
        moved = False
        for s, ps in sorted(assign[hi].items(), key=lambda kv: uls[kv[0]]):
            if not ps:
                continue
            w = uls[s] / 4
            dests = [c for c in range(NCORES)
                     if c != hi and (s in assign[c] or len(assign[c]) < SLOTS)]
            if not dests:
                continue
            lo = min(dests, key=lambda c: loads[c])
            if loads[lo] + w < loads[hi] - 1e-9:
                p = max(ps)
                ps.discard(p)
                if not ps:
                    del assign[hi][s]
                assign[lo].setdefault(s, set()).add(p)
                loads[hi] -= w
                loads[lo] += w
                moved = True
                break
        if not moved:
            break
    ranked = sorted(range(NCORES), key=lambda c: -loads[c])
    out = []
    for c in ranked:
        jobs = [(s, tuple(sorted(ps))) for s, ps in sorted(assign[c].items())
                if ps]
        out.append(jobs)
    return out


def _wrap_idx_units(units_row, L):
    """Per-unit wrapped int16 idx blocks: unit u's rows re-based at its
    row_lo, idx k at [k % 16, u*IDXC + k // 16]; pad with 0. The
    16-partition pattern is replicated to all 8 gpsimd cores."""
    arr = np.zeros((16, NGRP * IDXC), dtype=np.int16)
    units, _ = _units_for(L)
    v = units_row.astype(np.int16)
    for u, (rlo, rhi) in enumerate(units):
        n = rhi - rlo
        k = np.arange(n)
        arr[k % 16, u * IDXC + k // 16] = v[rlo:rhi]
    return np.tile(arr, (8, 1))


def _run(inputs, trace=False, tmpdir=None):
    centroids = np.ascontiguousarray(np.asarray(inputs["centroids"]), dtype=np.float32)
    units = np.asarray(inputs["units"])
    unit_lengths = np.asarray(inputs["unit_lengths"]).astype(np.int64)
    C = np.ascontiguousarray(np.asarray(inputs["C"]), dtype=np.float32)
    assert centroids.shape == (B, T, D) and C.shape == (K, D)

    assign = _plan(unit_lengths)
    core_jobs = tuple(
        tuple((int(unit_lengths[s]), pairs) for s, pairs in jobs)
        for jobs in assign
    )
    nc = _get_program(core_jobs)

    in_maps = []
    for jobs in assign:
        cent = np.empty((SLOTS, T, D), dtype=np.float32)
        idx = np.zeros((SLOTS, 128, NGRP * IDXC), dtype=np.int16)
        for slot, (s, _pairs) in enumerate(jobs):
            cent[slot] = centroids[s]
            idx[slot] = _wrap_idx_units(units[s], int(unit_lengths[s]))
        in_maps.append({"cent": cent, "cmat": C, "idx": idx})

    res = run_bass_kernel_spmd(nc, in_maps, list(range(NCORES)),
                               trace=trace, tmpdir=tmpdir)

    per_sample = np.zeros(B, dtype=np.float64)
    for core, jobs in enumerate(assign):
        sums = res.results[core]["out"].astype(np.float64)
        for slot, (s, _pairs) in enumerate(jobs):
            per_sample[s] += sums[:, slot].sum()
    total = float((per_sample / unit_lengths.astype(np.float64)).sum())
    return np.float32(total / B), res


def kernel(**inputs):
    out, _ = _run(inputs)
    return out
